# revision 19
# baseline (speedup 1.0000x reference)
"""MinGRU Trainium2 kernel.

Reference computation (B=8, T=2048, D=2048, fp32):
    z       = sigmoid(x @ Wz.T + bz)          [B,T,D]
    h_tilde = x @ Wh.T + bh                   [B,T,D]
    h_t     = (1-z_t) * h_{t-1} + z_t * h_tilde_t   (scan over T, h_0 = h_prev)
    returns (h, h[:, -1, :])

Strategy: data-parallel over batch, one batch per NeuronCore (8 cores).
Per core, everything is laid out [D-on-partitions, T-on-free]:
  - host pre-transposes x[b] -> xT [D,T] (bf16) and packs Wz.T/Wh.T per
    output-column chunk as [NE, 128, ND*128] bf16 so each e-chunk's weights
    load with a single contiguous-row DMA (the SP sequencer costs ~0.6us per
    DMA trigger, so trigger count matters),
  - PE computes P.T[e,t] = sum_d W[e,d] x[t,d] via lhsT=W-tile, rhs=xT-tile,
    accumulating over the 16 d-chunks in PSUM (bf16 in, fp32 accumulate),
  - ScalarE drains PSUM: a = sigmoid(-(pz+bz)) and z = sigmoid(pz+bz) with the
    per-partition bias fused into the activation,
  - VectorE forms b = (ph+bh)*z in one scalar_tensor_tensor op, then runs the
    recurrence state = a*state + b with tensor_tensor_scan along T (chained in
    four 512 chunks so the tail overlaps output DMA),
  - result [D,T] fp32 is DMA'd out and the host transposes back to [T,D].

e=0 interleaves the z/h projections per d-chunk so the PE only needs each
x-tile at half rate while the initial x DMAs stream in; later e-chunks run
proj-major so PSUM drains overlap the opposite projection's matmuls.
"""
import sys

if "/opt/trn_rl_repo" not in sys.path:
    sys.path.insert(0, "/opt/trn_rl_repo")

import numpy as np

B, T, D = 8, 2048, 2048
N_CORES = 8
PB = 128          # partition block
NE = D // PB      # 16 output-column chunks
ND = D // PB      # 16 contraction chunks
TT = 512          # matmul free-dim tile (one PSUM bank of fp32)
NT = T // TT      # 4 t-tiles


def _install_tail_patch():
    """The stock Tile kernel tail attaches one final wait per live semaphore
    (~300 here) to a single SP drain; generate_event_semaphores then expands
    them into a ~10us serial chain on one engine.  Distribute the final waits
    round-robin over all five engines so they resolve in parallel."""
    import concourse.mybir as mybir
    import concourse.tile as tile_mod
    from concourse.vector_clock import ScopedClock

    def _patched(self, tick_clock, wait_clock):
        nc = self.nc
        anchor = nc.sync.nop(nofuse=True, hint="tail_waits")
        wait_clock.add_sem_waits(
            anchor.ins, ScopedClock({None: tick_clock.global_clock})
        )
        waits = list(anchor.ins.sync_info.on_wait or [])
        engines = [nc.sync, nc.scalar, nc.vector, nc.tensor, nc.gpsimd]
        n = len(engines)
        anchor.ins.sync_info.on_wait = waits[0::n]
        for i, eng in enumerate(engines[1:], start=1):
            chunk = waits[i::n]
            if not chunk:
                continue
            w = eng.nop(nofuse=True, hint=f"tail_waits_{i}")
            wsi = w.ins.sync_info
            if wsi is None:
                wsi = mybir.SyncInfo(on_wait=[], on_update=[])
                w.ins.sync_info = wsi
            wsi.on_wait = chunk
        nc.sync.drain()
        nc.all_engine_barrier()
        assert self.sems is not None
        popped = nc._tile_sem_poison_stack.pop()
        assert popped is self._sem_poison
        nc.clear_and_free_semaphores(list(self.sems.allocated().values()))
        nc.all_engine_barrier()

    tile_mod.TileContext._drain_and_barrier = _patched


def build_program():
    import concourse.bass as bass
    import concourse.mybir as mybir
    import concourse.tile as tile

    _install_tail_patch()

    f32 = mybir.dt.float32
    bf16 = mybir.dt.bfloat16

    nc = bass.Bass("TRN2", target_bir_lowering=False, debug=False,
                   num_devices=N_CORES, enable_partition_id=False)

    xT = nc.dram_tensor("xT", [D, T], bf16, kind="ExternalInput")
    wz = nc.dram_tensor("wz", [NE, PB, ND * PB], bf16, kind="ExternalInput")
    wh = nc.dram_tensor("wh", [NE, PB, ND * PB], bf16, kind="ExternalInput")
    bzc = nc.dram_tensor("bzc", [PB, NE], f32, kind="ExternalInput")
    nbzc = nc.dram_tensor("nbzc", [PB, NE], f32, kind="ExternalInput")
    bhc = nc.dram_tensor("bhc", [PB, NE], f32, kind="ExternalInput")
    hpc = nc.dram_tensor("hpc", [PB, NE], f32, kind="ExternalInput")
    outT = nc.dram_tensor("outT", [D, T], f32, kind="ExternalOutput")

    sig = mybir.ActivationFunctionType.Sigmoid
    mult = mybir.AluOpType.mult
    add = mybir.AluOpType.add

    with tile.TileContext(nc) as tc:
        with (
            tc.tile_pool(name="singles", bufs=1) as singles,
            tc.tile_pool(name="xp", bufs=1) as xpool,
            tc.tile_pool(name="wp", bufs=2) as wpool,
            tc.tile_pool(name="zp", bufs=1, space="PSUM") as zpsum,
            tc.tile_pool(name="hp", bufs=1, space="PSUM") as hpsum,
            tc.tile_pool(name="ap", bufs=2) as apool,
            tc.tile_pool(name="zsb", bufs=2) as zspool,
            tc.tile_pool(name="bp", bufs=2) as bpool,
            tc.tile_pool(name="op", bufs=2) as opool,
        ):
            # DMA issue is split over two engines: SP (sync/HWDGE) handles
            # weights + output, GpSimd (SWDGE) handles x + biases, so their
            # ~0.6us-per-trigger issue costs run in parallel.  Chunk only the
            # latency-critical e=0 weight blocks and first x tiles.
            WCH = 4 * PB  # one weight chunk covers 4 d-chunks

            def load_weights(dst, src_e, nchunks):
                ch = ND * PB // nchunks
                for c in range(0, ND * PB, ch):
                    nc.sync.dma_start(out=dst[:, c:c + ch],
                                      in_=src_e[:, c:c + ch])

            wz0 = wpool.tile([PB, ND * PB], bf16, tag="wz")
            wh0 = wpool.tile([PB, ND * PB], bf16, tag="wh")

            x_tiles = [xpool.tile([PB, T], bf16, tag=f"x{d}", name=f"x_{d}")
                       for d in range(ND)]

            def load_x(d, nchunks):
                xt = x_tiles[d]
                ch = T // nchunks
                for t in range(nchunks):
                    nc.sync.dma_start(
                        out=xt[:, t * ch:(t + 1) * ch],
                        in_=xT[d * PB:(d + 1) * PB, t * ch:(t + 1) * ch])

            def load_w0_chunk(c):
                nc.sync.dma_start(out=wz0[:, c * WCH:(c + 1) * WCH],
                                  in_=wz[0][:, c * WCH:(c + 1) * WCH])
                nc.sync.dma_start(out=wh0[:, c * WCH:(c + 1) * WCH],
                                  in_=wh[0][:, c * WCH:(c + 1) * WCH])

            # Trigger order tracks first-use time: w chunk c is needed at
            # d-chunk 4c, x tile d at ~1.7us*d into e=0.
            nc.sync.dma_start(out=wz0[:, 0:WCH], in_=wz[0][:, 0:WCH])
            nc.sync.dma_start(out=x_tiles[0][:, 0:TT], in_=xT[0:PB, 0:TT])
            nc.sync.dma_start(out=wh0[:, 0:WCH], in_=wh[0][:, 0:WCH])
            nc.sync.dma_start(out=x_tiles[0][:, TT:T], in_=xT[0:PB, TT:T])
            load_x(1, 2)
            load_w0_chunk(1)
            load_x(2, 1)
            load_x(3, 1)
            load_w0_chunk(2)
            load_x(4, 1)
            load_x(5, 1)
            load_w0_chunk(3)
            for d in range(6, ND):
                load_x(d, 1)

            bz_sb = singles.tile([PB, NE], f32, tag="bz")
            nbz_sb = singles.tile([PB, NE], f32, tag="nbz")
            bh_sb = singles.tile([PB, NE], f32, tag="bh")
            hp_sb = singles.tile([PB, NE], f32, tag="hp")
            nc.sync.dma_start(out=bz_sb[:], in_=bzc[:])
            nc.sync.dma_start(out=nbz_sb[:], in_=nbzc[:])
            nc.sync.dma_start(out=bh_sb[:], in_=bhc[:])
            nc.sync.dma_start(out=hp_sb[:], in_=hpc[:])

            for e in range(NE):
                if e == 0:
                    wz_sb, wh_sb = wz0, wh0
                else:
                    wz_sb = wpool.tile([PB, ND * PB], bf16, tag="wz",
                                       name=f"wz_{e}")
                    load_weights(wz_sb, wz[e], 2)
                    wh_sb = wpool.tile([PB, ND * PB], bf16, tag="wh",
                                       name=f"wh_{e}")
                    load_weights(wh_sb, wh[e], 2)

                pz = [zpsum.tile([PB, TT], f32, name=f"pz_{e}_{t}",
                                 tag=f"pz{t}") for t in range(NT)]
                ph = [hpsum.tile([PB, TT], f32, name=f"ph_{e}_{t}",
                                 tag=f"ph{t}") for t in range(NT)]

                def mm_pass(psums, w_sb, d):
                    wsl = w_sb[:, d * PB:(d + 1) * PB]
                    for t in range(NT):
                        nc.tensor.matmul(
                            psums[t][:],
                            wsl,
                            x_tiles[d][:, t * TT:(t + 1) * TT],
                            start=(d == 0),
                            stop=(d == ND - 1),
                        )

                if e == 0:
                    # Interleave z/h per d-chunk: the PE only needs each x
                    # tile at half rate while the initial x DMAs stream in.
                    for d in range(ND):
                        mm_pass(pz, wz_sb, d)
                        mm_pass(ph, wh_sb, d)
                elif e == NE - 1:
                    # t-outer so each PSUM tile's accumulation finishes
                    # progressively and the drain/scan chain overlaps the
                    # final matmuls instead of trailing them.
                    for psums, w_sb in ((pz, wz_sb), (ph, wh_sb)):
                        for t in range(NT):
                            for d in range(ND):
                                nc.tensor.matmul(
                                    psums[t][:],
                                    w_sb[:, d * PB:(d + 1) * PB],
                                    x_tiles[d][:, t * TT:(t + 1) * TT],
                                    start=(d == 0),
                                    stop=(d == ND - 1),
                                )
                else:
                    for d in range(ND):
                        mm_pass(pz, wz_sb, d)
                    for d in range(ND):
                        mm_pass(ph, wh_sb, d)

                a_sb = apool.tile([PB, T], f32, tag="a", name=f"a_{e}")
                z_sb = zspool.tile([PB, T], f32, tag="z", name=f"z_{e}")
                b_sb = bpool.tile([PB, T], f32, tag="b", name=f"b_{e}")
                o_sb = opool.tile([PB, T], f32, tag="o", name=f"o_{e}")
                ecol = slice(e, e + 1)
                for t in range(NT):
                    sl = slice(t * TT, (t + 1) * TT)
                    nc.scalar.activation(a_sb[:, sl], pz[t][:], sig,
                                         bias=nbz_sb[:, ecol], scale=-1.0)
                    nc.scalar.activation(z_sb[:, sl], pz[t][:], sig,
                                         bias=bz_sb[:, ecol], scale=1.0)
                    nc.vector.scalar_tensor_tensor(
                        b_sb[:, sl], ph[t][:], bh_sb[:, ecol], z_sb[:, sl],
                        op0=add, op1=mult)
                for t in range(NT):
                    sl = slice(t * TT, (t + 1) * TT)
                    init = (hp_sb[:, ecol] if t == 0
                            else o_sb[:, t * TT - 1:t * TT])
                    nc.vector.tensor_tensor_scan(
                        o_sb[:, sl], a_sb[:, sl], b_sb[:, sl], init,
                        op0=mult, op1=add)
                    nc.sync.dma_start(
                        out=outT[e * PB:(e + 1) * PB, t * TT:(t + 1) * TT],
                        in_=o_sb[:, sl])

    # This walrus build accepts at most one sync wait per instruction;
    # move_matmul_waits_to_ldweights + generate_event_semaphores split any
    # multi-wait instructions the Tile scheduler emitted.
    import bass_rust as _bass_rust

    _bass_rust.move_matmul_waits_to_ldweights(nc.m)
    _bass_rust.generate_event_semaphores(nc)
    return nc


def prepare_inputs(x, h_prev, Wz, bz, Wh, bh):
    import ml_dtypes

    bf16 = ml_dtypes.bfloat16
    x = np.asarray(x, dtype=np.float32)
    h_prev = np.asarray(h_prev, dtype=np.float32)
    Wz = np.asarray(Wz, dtype=np.float32)
    Wh = np.asarray(Wh, dtype=np.float32)
    bz = np.asarray(bz, dtype=np.float32)
    bh = np.asarray(bh, dtype=np.float32)

    # Packed so that for each e-chunk, partition p holds the contiguous run
    # over (d-chunk, within-d) of W.T[d*128+p, e*128+q]:
    #   pack[e, p, d*128+q] = W[e*128+q, d*128+p]
    def pack(W):
        return np.ascontiguousarray(
            W.reshape(NE, PB, ND, PB).transpose(0, 3, 2, 1)
        ).reshape(NE, PB, ND * PB).astype(bf16)

    wz_p = pack(Wz)
    wh_p = pack(Wh)
    bz_c = np.ascontiguousarray(bz.reshape(NE, PB).T)
    nbz_c = np.ascontiguousarray((-bz).reshape(NE, PB).T)
    bh_c = np.ascontiguousarray(bh.reshape(NE, PB).T)

    in_maps = []
    for b in range(B):
        in_maps.append({
            "xT": np.ascontiguousarray(x[b].T).astype(bf16),
            "wz": wz_p,
            "wh": wh_p,
            "bzc": bz_c,
            "nbzc": nbz_c,
            "bhc": bh_c,
            "hpc": np.ascontiguousarray(h_prev[b].reshape(NE, PB).T),
        })
    return in_maps


def run(in_maps, trace=False):
    from concourse.bass_utils import run_bass_kernel_spmd

    nc = build_program()
    return run_bass_kernel_spmd(nc, in_maps, list(range(N_CORES)), trace=trace)


def assemble_output(results):
    out = np.empty((B, T, D), dtype=np.float32)
    for b in range(B):
        out[b] = results[b]["outT"].T
    return out, np.ascontiguousarray(out[:, -1, :])


def kernel(x, h_prev, Wz, bz, Wh, bh):
    in_maps = prepare_inputs(x, h_prev, Wz, bz, Wh, bh)
    res = run(in_maps, trace=False)
    return assemble_output(res.results)


# revision 20
# speedup vs baseline: 1.0040x; 1.0040x over previous
"""MinGRU Trainium2 kernel.

Reference computation (B=8, T=2048, D=2048, fp32):
    z       = sigmoid(x @ Wz.T + bz)          [B,T,D]
    h_tilde = x @ Wh.T + bh                   [B,T,D]
    h_t     = (1-z_t) * h_{t-1} + z_t * h_tilde_t   (scan over T, h_0 = h_prev)
    returns (h, h[:, -1, :])

Strategy: data-parallel over batch, one batch per NeuronCore (8 cores).
Per core, everything is laid out [D-on-partitions, T-on-free]:
  - host pre-transposes x[b] -> xT [D,T] (bf16) and packs Wz.T/Wh.T per
    output-column chunk as [NE, 128, ND*128] bf16 so each e-chunk's weights
    load with a single contiguous-row DMA (the SP sequencer costs ~0.6us per
    DMA trigger, so trigger count matters),
  - PE computes P.T[e,t] = sum_d W[e,d] x[t,d] via lhsT=W-tile, rhs=xT-tile,
    accumulating over the 16 d-chunks in PSUM (bf16 in, fp32 accumulate),
  - ScalarE drains PSUM: a = sigmoid(-(pz+bz)) and z = sigmoid(pz+bz) with the
    per-partition bias fused into the activation,
  - VectorE forms b = (ph+bh)*z in one scalar_tensor_tensor op, then runs the
    recurrence state = a*state + b with tensor_tensor_scan along T (chained in
    four 512 chunks so the tail overlaps output DMA),
  - result [D,T] fp32 is DMA'd out and the host transposes back to [T,D].

e=0 interleaves the z/h projections per d-chunk so the PE only needs each
x-tile at half rate while the initial x DMAs stream in; later e-chunks run
proj-major so PSUM drains overlap the opposite projection's matmuls.
"""
import sys

if "/opt/trn_rl_repo" not in sys.path:
    sys.path.insert(0, "/opt/trn_rl_repo")

import numpy as np

B, T, D = 8, 2048, 2048
N_CORES = 8
PB = 128          # partition block
NE = D // PB      # 16 output-column chunks
ND = D // PB      # 16 contraction chunks
TT = 512          # matmul free-dim tile (one PSUM bank of fp32)
NT = T // TT      # 4 t-tiles


def _install_tail_patch():
    """The stock Tile kernel tail attaches one final wait per live semaphore
    (~300 here) to a single SP drain; generate_event_semaphores then expands
    them into a ~10us serial chain on one engine.  Distribute the final waits
    round-robin over all five engines so they resolve in parallel."""
    import concourse.mybir as mybir
    import concourse.tile as tile_mod
    from concourse.vector_clock import ScopedClock

    def _patched(self, tick_clock, wait_clock):
        nc = self.nc
        anchor = nc.sync.nop(nofuse=True, hint="tail_waits")
        wait_clock.add_sem_waits(
            anchor.ins, ScopedClock({None: tick_clock.global_clock})
        )
        waits = list(anchor.ins.sync_info.on_wait or [])
        engines = [nc.sync, nc.scalar, nc.vector, nc.tensor, nc.gpsimd]
        n = len(engines)
        anchor.ins.sync_info.on_wait = waits[0::n]
        for i, eng in enumerate(engines[1:], start=1):
            chunk = waits[i::n]
            if not chunk:
                continue
            w = eng.nop(nofuse=True, hint=f"tail_waits_{i}")
            wsi = w.ins.sync_info
            if wsi is None:
                wsi = mybir.SyncInfo(on_wait=[], on_update=[])
                w.ins.sync_info = wsi
            wsi.on_wait = chunk
        nc.sync.drain()
        nc.all_engine_barrier()
        assert self.sems is not None
        popped = nc._tile_sem_poison_stack.pop()
        assert popped is self._sem_poison
        nc.clear_and_free_semaphores(list(self.sems.allocated().values()))
        nc.all_engine_barrier()

    tile_mod.TileContext._drain_and_barrier = _patched


def build_program():
    import concourse.bass as bass
    import concourse.mybir as mybir
    import concourse.tile as tile

    _install_tail_patch()

    f32 = mybir.dt.float32
    bf16 = mybir.dt.bfloat16

    nc = bass.Bass("TRN2", target_bir_lowering=False, debug=False,
                   num_devices=N_CORES, enable_partition_id=False)

    xT = nc.dram_tensor("xT", [D, T], bf16, kind="ExternalInput")
    wz = nc.dram_tensor("wz", [NE, PB, ND * PB], bf16, kind="ExternalInput")
    wh = nc.dram_tensor("wh", [NE, PB, ND * PB], bf16, kind="ExternalInput")
    bzc = nc.dram_tensor("bzc", [PB, NE], f32, kind="ExternalInput")
    nbzc = nc.dram_tensor("nbzc", [PB, NE], f32, kind="ExternalInput")
    bhc = nc.dram_tensor("bhc", [PB, NE], f32, kind="ExternalInput")
    hpc = nc.dram_tensor("hpc", [PB, NE], f32, kind="ExternalInput")
    outT = nc.dram_tensor("outT", [D, T], f32, kind="ExternalOutput")

    sig = mybir.ActivationFunctionType.Sigmoid
    mult = mybir.AluOpType.mult
    add = mybir.AluOpType.add

    with tile.TileContext(nc) as tc:
        with (
            tc.tile_pool(name="singles", bufs=1) as singles,
            tc.tile_pool(name="xp", bufs=1) as xpool,
            tc.tile_pool(name="wp", bufs=2) as wpool,
            tc.tile_pool(name="zp", bufs=1, space="PSUM") as zpsum,
            tc.tile_pool(name="hp", bufs=1, space="PSUM") as hpsum,
            tc.tile_pool(name="ap", bufs=2) as apool,
            tc.tile_pool(name="zsb", bufs=2) as zspool,
            tc.tile_pool(name="bp", bufs=2) as bpool,
            tc.tile_pool(name="op", bufs=2) as opool,
        ):
            # DMA issue is split over two engines: SP (sync/HWDGE) handles
            # weights + output, GpSimd (SWDGE) handles x + biases, so their
            # ~0.6us-per-trigger issue costs run in parallel.  Chunk only the
            # latency-critical e=0 weight blocks and first x tiles.
            WCH = 4 * PB  # one weight chunk covers 4 d-chunks

            def load_weights(dst, src_e, nchunks):
                ch = ND * PB // nchunks
                for c in range(0, ND * PB, ch):
                    nc.sync.dma_start(out=dst[:, c:c + ch],
                                      in_=src_e[:, c:c + ch])

            wz0 = wpool.tile([PB, ND * PB], bf16, tag="wz")
            wh0 = wpool.tile([PB, ND * PB], bf16, tag="wh")

            x_tiles = [xpool.tile([PB, T], bf16, tag=f"x{d}", name=f"x_{d}")
                       for d in range(ND)]

            def load_x(d, nchunks):
                xt = x_tiles[d]
                ch = T // nchunks
                for t in range(nchunks):
                    nc.sync.dma_start(
                        out=xt[:, t * ch:(t + 1) * ch],
                        in_=xT[d * PB:(d + 1) * PB, t * ch:(t + 1) * ch])

            def load_w0_chunk(c):
                nc.sync.dma_start(out=wz0[:, c * WCH:(c + 1) * WCH],
                                  in_=wz[0][:, c * WCH:(c + 1) * WCH])
                nc.sync.dma_start(out=wh0[:, c * WCH:(c + 1) * WCH],
                                  in_=wh[0][:, c * WCH:(c + 1) * WCH])

            # Trigger order tracks first-use time: w chunk c is needed at
            # d-chunk 4c, x tile d at ~1.7us*d into e=0.
            nc.sync.dma_start(out=wz0[:, 0:WCH], in_=wz[0][:, 0:WCH])
            nc.sync.dma_start(out=x_tiles[0][:, 0:TT], in_=xT[0:PB, 0:TT])
            nc.sync.dma_start(out=wh0[:, 0:WCH], in_=wh[0][:, 0:WCH])
            nc.sync.dma_start(out=x_tiles[0][:, TT:T], in_=xT[0:PB, TT:T])
            load_x(1, 2)
            load_w0_chunk(1)
            load_x(2, 1)
            load_x(3, 1)
            load_w0_chunk(2)
            load_x(4, 1)
            load_x(5, 1)
            load_w0_chunk(3)
            for d in range(6, ND):
                load_x(d, 1)

            bz_sb = singles.tile([PB, NE], f32, tag="bz")
            nbz_sb = singles.tile([PB, NE], f32, tag="nbz")
            bh_sb = singles.tile([PB, NE], f32, tag="bh")
            hp_sb = singles.tile([PB, NE], f32, tag="hp")
            nc.sync.dma_start(out=bz_sb[:], in_=bzc[:])
            nc.sync.dma_start(out=nbz_sb[:], in_=nbzc[:])
            nc.sync.dma_start(out=bh_sb[:], in_=bhc[:])
            nc.sync.dma_start(out=hp_sb[:], in_=hpc[:])

            for e in range(NE):
                if e == 0:
                    wz_sb, wh_sb = wz0, wh0
                else:
                    wz_sb = wpool.tile([PB, ND * PB], bf16, tag="wz",
                                       name=f"wz_{e}")
                    load_weights(wz_sb, wz[e], 2)
                    wh_sb = wpool.tile([PB, ND * PB], bf16, tag="wh",
                                       name=f"wh_{e}")
                    load_weights(wh_sb, wh[e], 2)

                pz = [zpsum.tile([PB, TT], f32, name=f"pz_{e}_{t}",
                                 tag=f"pz{t}") for t in range(NT)]
                ph = [hpsum.tile([PB, TT], f32, name=f"ph_{e}_{t}",
                                 tag=f"ph{t}") for t in range(NT)]

                def mm_pass(psums, w_sb, d):
                    wsl = w_sb[:, d * PB:(d + 1) * PB]
                    for t in range(NT):
                        nc.tensor.matmul(
                            psums[t][:],
                            wsl,
                            x_tiles[d][:, t * TT:(t + 1) * TT],
                            start=(d == 0),
                            stop=(d == ND - 1),
                        )

                if e == 0:
                    # Interleave z/h per d-chunk: the PE only needs each x
                    # tile at half rate while the initial x DMAs stream in.
                    for d in range(ND):
                        mm_pass(pz, wz_sb, d)
                        mm_pass(ph, wh_sb, d)
                elif e == NE - 1:
                    # t-outer so each PSUM tile's accumulation finishes
                    # progressively and the drain/scan chain overlaps the
                    # final matmuls instead of trailing them.
                    for psums, w_sb in ((pz, wz_sb), (ph, wh_sb)):
                        for t in range(NT):
                            for d in range(ND):
                                nc.tensor.matmul(
                                    psums[t][:],
                                    w_sb[:, d * PB:(d + 1) * PB],
                                    x_tiles[d][:, t * TT:(t + 1) * TT],
                                    start=(d == 0),
                                    stop=(d == ND - 1),
                                )
                else:
                    for d in range(ND):
                        mm_pass(pz, wz_sb, d)
                    for d in range(ND):
                        mm_pass(ph, wh_sb, d)

                a_sb = apool.tile([PB, T], f32, tag="a", name=f"a_{e}")
                z_sb = zspool.tile([PB, T], f32, tag="z", name=f"z_{e}")
                b_sb = bpool.tile([PB, T], f32, tag="b", name=f"b_{e}")
                o_sb = opool.tile([PB, T], f32, tag="o", name=f"o_{e}")
                ecol = slice(e, e + 1)
                for t in range(NT):
                    sl = slice(t * TT, (t + 1) * TT)
                    nc.scalar.activation(a_sb[:, sl], pz[t][:], sig,
                                         bias=nbz_sb[:, ecol], scale=-1.0)
                    nc.scalar.activation(z_sb[:, sl], pz[t][:], sig,
                                         bias=bz_sb[:, ecol], scale=1.0)
                    nc.vector.scalar_tensor_tensor(
                        b_sb[:, sl], ph[t][:], bh_sb[:, ecol], z_sb[:, sl],
                        op0=add, op1=mult)
                for t in range(NT):
                    sl = slice(t * TT, (t + 1) * TT)
                    init = (hp_sb[:, ecol] if t == 0
                            else o_sb[:, t * TT - 1:t * TT])
                    nc.vector.tensor_tensor_scan(
                        o_sb[:, sl], a_sb[:, sl], b_sb[:, sl], init,
                        op0=mult, op1=add)
                    nc.sync.dma_start(
                        out=outT[e * PB:(e + 1) * PB, t * TT:(t + 1) * TT],
                        in_=o_sb[:, sl])

    # This walrus build accepts at most one sync wait per instruction;
    # move_matmul_waits_to_ldweights + generate_event_semaphores split any
    # multi-wait instructions the Tile scheduler emitted.
    import bass_rust as _bass_rust

    _bass_rust.move_matmul_waits_to_ldweights(nc.m)
    _bass_rust.generate_event_semaphores(nc)
    return nc


def prepare_inputs(x, h_prev, Wz, bz, Wh, bh):
    import ml_dtypes

    bf16 = ml_dtypes.bfloat16
    x = np.asarray(x, dtype=np.float32)
    h_prev = np.asarray(h_prev, dtype=np.float32)
    Wz = np.asarray(Wz, dtype=np.float32)
    Wh = np.asarray(Wh, dtype=np.float32)
    bz = np.asarray(bz, dtype=np.float32)
    bh = np.asarray(bh, dtype=np.float32)

    # Packed so that for each e-chunk, partition p holds the contiguous run
    # over (d-chunk, within-d) of W.T[d*128+p, e*128+q]:
    #   pack[e, p, d*128+q] = W[e*128+q, d*128+p]
    def pack(W):
        return np.ascontiguousarray(
            W.reshape(NE, PB, ND, PB).transpose(0, 3, 2, 1)
        ).reshape(NE, PB, ND * PB).astype(bf16)

    wz_p = pack(Wz)
    wh_p = pack(Wh)
    bz_c = np.ascontiguousarray(bz.reshape(NE, PB).T)
    nbz_c = np.ascontiguousarray((-bz).reshape(NE, PB).T)
    bh_c = np.ascontiguousarray(bh.reshape(NE, PB).T)

    in_maps = []
    for b in range(B):
        in_maps.append({
            "xT": np.ascontiguousarray(x[b].T).astype(bf16),
            "wz": wz_p,
            "wh": wh_p,
            "bzc": bz_c,
            "nbzc": nbz_c,
            "bhc": bh_c,
            "hpc": np.ascontiguousarray(h_prev[b].reshape(NE, PB).T),
        })
    return in_maps


def run(in_maps, trace=False):
    from concourse.bass_utils import run_bass_kernel_spmd

    nc = build_program()
    return run_bass_kernel_spmd(nc, in_maps, list(range(N_CORES)), trace=trace)


def assemble_output(results):
    out = np.empty((B, T, D), dtype=np.float32)
    for b in range(B):
        out[b] = results[b]["outT"].T
    return out, np.ascontiguousarray(out[:, -1, :])


def kernel(x, h_prev, Wz, bz, Wh, bh):
    in_maps = prepare_inputs(x, h_prev, Wz, bz, Wh, bh)
    last_err = None
    for _ in range(3):
        try:
            res = run(in_maps, trace=False)
            return assemble_output(res.results)
        except Exception as e:  # transient axon/PJRT hiccups
            last_err = e
    raise last_err
